# revision 29
# baseline (speedup 1.0000x reference)
"""ArDCA pseudo-likelihood loss on 8 Trainium2 NeuronCores.

Math (reference): for samples X (M,L) over alphabet Q with weights W,
    pair[m,i,a] = sum_{j<i} J[i,j,a,X[m,j]]
    logits = h_pos + pair ;  loss = -sum_{m,i} W[m]*log_softmax(logits)[gold]
                              + lam_h*|h|^2 + lam_j*|tril(J)|^2

Strategy: data-parallel over M (1024 samples/core).  The one-hot einsum is a
dense TensorEngine matmul: out[m, (i,a)] += onehotT[(j,b), m].T @ J[(j,b),(i,a)]
with K = L*Q = 5376 contraction packed 6 j-positions per 128-row K-tile and the
strict lower-triangle (j<i) skipped at column granularity.  J (tril-masked,
scaled by 64, fp8-e4m3) stays SBUF-resident and streams through the PE as fp8
DoubleRow pairs; h_pos rides in as a bias row of K-tile 0 against an all-ones
row of the one-hot.  The PE streams one output column per 2.4GHz cycle
(~200us/core of column traffic) and is the roofline; everything else is
arranged to keep it fed:

  * J is packed in two regions matching consumption order (wave-1 = i-blocks
    0..7, wave-2 = 8..10) and DMA'd in ~20 large chunks issued from the
    otherwise-idle Sync queue (a dma_start costs ~670ns of issue time
    regardless of size; the old 95-transfer scheme serialized ~63us of issue
    and starved the PE for the first 100us).  The K-oriented one-hot is
    shipped partition-major so chunked transfers stay 2D.
  * Epilogue per (m-tile, i-block): exp on ScalarE (scale=1/64 folded in,
    bf16 out, sole PSUM reader -> fastest bank recycle), one-hot build +
    gold product on DVE against SBUF bf16, exp-segment-reduce on the idle
    GpSimd engine, gold-segment-reduce on DVE.  Per m-tile two Ln+accum ACTs
    turn the segment sums into sum_i ln Z_i and sum_i z_gold (ln of exp).
  * regJ/regH are tiny constants of the inputs; they are computed host-side
    in float64 and added to the device NLL.

Known landmines on this stack (found the hard way): tensor_tensor_reduce and
any fp8 operand on the VectorEngine hard-crash the device; engine APs must
start at 32-aligned partitions; interleaving Exp/Ln/Square per-tile thrashes
ACT table sets (~1.3us per reload).
"""

import os
import sys

import numpy as np
import ml_dtypes

try:
    import concourse.bass as bass  # noqa: F401
except ImportError:  # pragma: no cover
    sys.path.insert(0, "/opt/trn_rl_repo")

import concourse.bass as bass
import concourse.mybir as mybir
import concourse.tile as tile
from concourse import bacc
from concourse.bass_utils import run_bass_kernel_spmd

# ---------------------------------------------------------------- constants
M, L, Q = 8192, 256, 21
LAMBDA_H = 1e-06
LAMBDA_J = 0.0001

NCORES = 8
MC = M // NCORES        # 1024 samples per core
MT = MC // 128          # 8 m-tiles per core
LQ = L * Q              # 5376

JPK = 6                 # j-positions per K-tile (6*21=126 <= 128)
KT = (L + JPK - 1) // JPK   # 43 K-tiles
IB = 24                 # i-positions per i-block (24*21=504 <= 512 psum bank)
NIB = (L + IB - 1) // IB    # 11 i-blocks (10 of 24 + 1 of 16)
IB_N = [min(IB, L - IB * b) * Q for b in range(NIB)]  # 504 .. 336
SCALE = 64.0            # fp8 pre-scale on J / h
WSPLIT = 8              # i-blocks 0..7 in wave 1, 8..10 in wave 2

FP8 = ml_dtypes.float8_e4m3
BF16 = ml_dtypes.bfloat16

# first i-block each K-tile contributes to: need some i in block with i > 6*kt
BMIN = [(JPK * kt + 1) // IB for kt in range(KT)]
# last K-tile contributing to i-block b:  j <= i_max-1 = min(IB*(b+1),L)-2
LASTKT = [min(KT - 1, (IB * (b + 1) - 2) // JPK) for b in range(NIB)]


def _pad16(w):
    return (w + 15) // 16 * 16


# wave-region packed J column widths / offsets (per partition, in elements).
# Region 1 holds each K-tile's columns for i-blocks BMIN..7, region 2 for
# i-blocks max(8,BMIN)..10.  Widths are padded to 16 so the DoubleRow
# middle-dim step (= width of the even K-tile of a pair) meets the ISA's
# step%16 rule.
W1REAL = [504 * max(0, WSPLIT - BMIN[kt]) for kt in range(KT)]
W2REAL = [
    sum(IB_N[b] for b in range(max(WSPLIT, BMIN[kt]), NIB)) for kt in range(KT)
]
JW1 = [_pad16(w) for w in W1REAL]
JW2 = [_pad16(w) for w in W2REAL]
JOFS1 = np.concatenate([[0], np.cumsum(JW1)]).astype(int)
TOT1 = int(JOFS1[-1])
JOFS2 = (TOT1 + np.concatenate([[0], np.cumsum(JW2)])).astype(int)
TOTW = int(JOFS2[-1])
NPAIR = KT // 2  # 21 DoubleRow pairs; K-tile 42 runs as a plain matmul

WAVES = [(0, WSPLIT), (WSPLIT, NIB)]

# DMA chunk plans: (kind, klo, khi) in issue order on the Sync queue.  Sizes
# chosen so the first matmul's data (xo kt0-1 + J-w1 kt0-3) lands within a
# few us and the rest streams in consumption order.
_DMA_PLAN = [
    ("xo", 0, 2), ("j1", 0, 2), ("j1", 2, 4), ("xo", 2, 8), ("j1", 4, 8),
    ("xo", 8, 16), ("j1", 8, 12), ("j1", 12, 16),
    ("xo", 16, 24), ("j1", 16, 20), ("j1", 20, 24),
    ("xo", 24, 32), ("j1", 24, 28), ("j1", 28, 32),
    ("xo", 32, 43),
    ("j2", 0, 8), ("j2", 8, 16), ("j2", 16, 24), ("j2", 24, 32), ("j2", 32, 43),
]

_DT = mybir.dt


# ---------------------------------------------------------------- host prep
def _prep_shared(J, h_pos):
    """tril-mask, scale, transpose J into the wave-packed (128, TOTW) fp8."""
    J = np.asarray(J, np.float32)
    h = np.asarray(h_pos, np.float32)
    mask = np.tril(np.ones((L, L), np.float32), k=-1)
    out = np.zeros((128, TOTW), FP8)
    for kt in range(KT):
        j0 = JPK * kt
        jw = min(JPK, L - j0)
        blk = J[:, j0 : j0 + jw] * mask[:, j0 : j0 + jw, None, None]  # (i,j',a,b)
        t = blk.transpose(1, 3, 0, 2).reshape(jw * Q, LQ)  # rows=(j',b), cols=(i,a)
        tl = np.zeros((128, LQ), np.float32)
        tl[: jw * Q] = t
        if kt == 0:
            tl[126] = h.reshape(LQ)
        tl *= SCALE
        f8 = tl.astype(FP8)
        if W1REAL[kt]:
            c0 = 504 * BMIN[kt]
            out[:, JOFS1[kt] : JOFS1[kt] + W1REAL[kt]] = f8[:, c0 : c0 + W1REAL[kt]]
        if W2REAL[kt]:
            c0 = sum(IB_N[b] for b in range(max(WSPLIT, BMIN[kt])))
            c0 = 504 * max(WSPLIT, BMIN[kt])
            out[:, JOFS2[kt] : JOFS2[kt] + W2REAL[kt]] = f8[:, c0 : c0 + W2REAL[kt]]
    return out


def _prep_core(Xs, Ws):
    """Per-core one-hots (K- and m-orientation) + weight tile."""
    Xs = np.asarray(Xs)
    jj = np.arange(L)
    # K-oriented one-hot, partition-major: xoht[row, kt*MC + m];
    # row = 21*(j%6)+b, so chunked [128, cols] DMA slices stay 2D.
    xoht = np.zeros((128, KT * MC), np.float32)
    rows = Q * (jj % JPK)[None, :] + Xs  # (MC, L)
    kts = (jj // JPK)[None, :].repeat(MC, 0)  # (MC, L)
    mm = np.arange(MC)[:, None].repeat(L, 1)
    xoht[rows.ravel(), kts.ravel() * MC + mm.ravel()] = 1.0
    xoht[126, 0:MC] = 1.0  # bias row pairs with h row in J (K-tile 0)
    # m-oriented one-hot for the gold product: ohm[p, t*LQ + i*Q + a];
    # fp8 so it is read only by the Pool mult (DVE must never touch fp8)
    Xp = Xs.reshape(MT, 128, L).transpose(1, 0, 2)  # (128, MT, L)
    ohm = np.zeros((128, MT * L * Q), np.float32)
    pcol = (np.arange(MT * L) * Q)[None, :] + Xp.reshape(128, MT * L)
    ohm[np.arange(128)[:, None], pcol] = 1.0
    wt = np.ascontiguousarray(np.asarray(Ws, np.float32).reshape(MT, 128).T)
    return xoht.astype(FP8), ohm.astype(FP8), wt


def _host_reg(J, h_pos):
    """regH + regJ (tril) in float64 — tiny constants of the inputs."""
    J = np.asarray(J, np.float32)
    h = np.asarray(h_pos, np.float64)
    s = 0.0
    for i in range(L):
        s += float((J[i, :i].astype(np.float64) ** 2).sum())
    return LAMBDA_J * s + LAMBDA_H * float((h * h).sum())


# ---------------------------------------------------------------- device code
def _patch_act_tables():
    """Make the act-table-load pass place Exp and Ln in their combined set.

    The pass assigns each ACT the first table set containing its function:
    Exp -> exp_and_others, Ln -> natural_log, so per-m-tile Ln accumulation
    ping-pongs table loads (~1.3us each, 18 per kernel).  Hiding Exp/Ln from
    every set except natural_log_exp_and_others (which holds both at full
    400-segment precision) routes both to one resident set.  Set ids keep
    their canonical insertion order, so the emitted InstLoadActFuncSet ids
    still match the real act_info.json.
    """
    import concourse.bacc as _bacc
    from concourse.hw_specs import get_activation_tables as _orig

    if getattr(_bacc, "_ardca_act_patch", False):
        return

    def patched(arch):
        tables = _orig(arch)
        both = {mybir.ActivationFunctionType.Exp, mybir.ActivationFunctionType.Ln}
        out = {}
        for name, s in tables.items():
            if name != "natural_log_exp_and_others":
                s = s - both
            out[name] = s
        return out

    _bacc.get_activation_tables = patched
    _bacc._ardca_act_patch = True


def _build_graph(opts=None):
    _patch_act_tables()
    o = {
        "bufs": (4, 4, 3),            # epool, prpool, opool depths
        # wave-2 dues per wave-1 slot: lag 2 early (relaxes the deep-J DMA
        # deadline past the ~400GB/s arrival curve), catch up at t=6 so the
        # tail still ends on a single cheap wave-2
        "dues": {2: [0], 3: [1], 4: [2], 5: [3], 6: [4, 5], 7: [6]},
    }
    o.update(opts or {})
    EB, PB, OB = o["bufs"]
    nc = bacc.Bacc(
        "TRN2", target_bir_lowering=False, debug=False, num_devices=NCORES
    )
    jd = nc.dram_tensor("jrs", [128, TOTW], _DT.float8e4, kind="ExternalInput")
    xd = nc.dram_tensor("xoht", [128, KT * MC], _DT.float8e4, kind="ExternalInput")
    ohd = nc.dram_tensor("ohm", [128, MT * LQ], _DT.float8e4, kind="ExternalInput")
    wd = nc.dram_tensor("wt", [128, MT], _DT.float32, kind="ExternalInput")
    outd = nc.dram_tensor("out", [1, 1], _DT.float32, kind="ExternalOutput")

    f32, fp8, bf16 = _DT.float32, _DT.float8e4, _DT.bfloat16

    with tile.TileContext(nc) as tc:
        with (
            tc.tile_pool(name="jres", bufs=1) as jpool,
            tc.tile_pool(name="xres", bufs=1) as xpool,
            tc.tile_pool(name="consts", bufs=1) as cpool,
            tc.tile_pool(name="psum", bufs=8, space="PSUM") as ppool,
            tc.tile_pool(name="exps", bufs=EB) as epool,
            tc.tile_pool(name="prods", bufs=PB) as prpool,
            tc.tile_pool(name="ohms", bufs=OB) as opool,
            tc.tile_pool(name="small", bufs=2) as spool,
        ):
            jt = jpool.tile([128, TOTW], fp8)
            xo = xpool.tile([128, KT * MC], fp8)
            # chunked, consumption-ordered input stream on the Sync queue
            for kind, klo, khi in _DMA_PLAN:
                if kind == "xo":
                    nc.sync.dma_start(
                        xo[:, klo * MC : khi * MC], xd[:, klo * MC : khi * MC]
                    )
                elif kind == "j1":
                    nc.sync.dma_start(
                        jt[:, JOFS1[klo] : JOFS1[khi]],
                        jd[:, JOFS1[klo] : JOFS1[khi]],
                    )
                else:
                    nc.sync.dma_start(
                        jt[:, JOFS2[klo] : JOFS2[khi]],
                        jd[:, JOFS2[klo] : JOFS2[khi]],
                    )
            wt = cpool.tile([128, MT], f32)
            nc.scalar.dma_start(wt[:], wd[:])

            # S[m, t*L + i] = sum_a exp(logits[m,i,a]); G likewise for the
            # gold product (= exp(z_gold)).
            sbig = cpool.tile([128, MT * L], f32)
            gbig = cpool.tile([128, MT * L], f32)

            def fetch_ohm(t):
                # prefetch the m-oriented one-hot slab for m-tile t (fp8,
                # 5.25KB/partition); ring depth = opool bufs
                oh = opool.tile([128, LQ], fp8, tag="oh", name=f"ohm_{t}")
                nc.scalar.dma_start(oh[:], ohd[:, t * LQ : (t + 1) * LQ])
                return oh

            def epilogue(t, ib, ps, oh):
                w = IB_N[ib]
                nI = w // Q
                c0 = t * L + IB * ib
                # exp on ACT is the sole PSUM reader: bank releases here
                e = epool.tile([128, 504], bf16, tag="exp")
                nc.scalar.activation(
                    e[:, :w], ps[:, :w], mybir.ActivationFunctionType.Exp,
                    scale=1.0 / SCALE,
                )
                # gold product on the otherwise-idle GpSimd engine
                # (host-shipped fp8 one-hot; Pool, unlike DVE, tolerates fp8)
                c = IB * ib * Q
                pr = prpool.tile([128, 504], bf16, tag="pr")
                nc.gpsimd.tensor_tensor(
                    out=pr[:, :w], in0=e[:, :w], in1=oh[:, c : c + w],
                    op=mybir.AluOpType.mult,
                )
                # segment reduces (Z denominators + exp(z_gold) per i) on DVE
                nc.vector.reduce_sum(
                    sbig[:, c0 : c0 + nI],
                    e[:, :w].rearrange("p (i a) -> p i a", a=Q),
                    axis=mybir.AxisListType.X,
                )
                nc.vector.reduce_sum(
                    gbig[:, c0 : c0 + nI],
                    pr[:, :w].rearrange("p (i a) -> p i a", a=Q),
                    axis=mybir.AxisListType.X,
                )

            jt_pitch = jt[:].ap[0][0]
            xo_pitch = xo[:].ap[0][0]

            def jt3(c0, step1, w):
                return bass.AP(
                    jt.tensor, int(jt.offset + c0),
                    [[int(jt_pitch), 128], [int(step1), 2], [1, int(w)]],
                )

            def xo3(p, t):
                return bass.AP(
                    xo.tensor, int(xo.offset + 2 * p * MC + t * 128),
                    [[int(xo_pitch), 128], [MC, 2], [1, 128]],
                )

            def run_wave(t, ib_lo, ib_hi, oh):
                w2 = ib_lo >= WSPLIT
                jofs, jw = (JOFS2, JW2) if w2 else (JOFS1, JW1)
                base = (lambda kt: max(WSPLIT, BMIN[kt])) if w2 else BMIN.__getitem__
                psums = {}
                for ib in range(ib_lo, ib_hi):
                    psums[ib] = ppool.tile(
                        [128, 504], f32, tag="ps", name=f"ps_{t}_{ib}"
                    )
                pair_hi = (max(LASTKT[ib] for ib in range(ib_lo, ib_hi)) + 1) // 2
                for p in range(pair_hi):
                    kt = 2 * p
                    lhs = xo3(p, t)
                    for ib in range(max(ib_lo, BMIN[kt]), ib_hi):
                        if kt > LASTKT[ib]:
                            continue
                        w = IB_N[ib]
                        c0 = int(jofs[kt]) + 504 * (ib - base(kt))
                        # in the pair's first i-block, columns i <= 12p are
                        # fully masked — skip streaming them.  Pair 0 must
                        # stay full width: it carries the h bias row (valid
                        # for every i) and the start=True PSUM clear.
                        off = 0
                        if p > 0 and ib == BMIN[kt]:
                            off = max(0, (JPK * kt + 1 - IB * ib)) * Q
                        nc.tensor.matmul(
                            psums[ib][:, off:w],
                            lhs,
                            jt3(c0 + off, jw[kt], w - off),
                            start=(p == 0),
                            stop=(kt + 1 == LASTKT[ib]),
                            perf_mode=mybir.MatmulPerfMode.DoubleRow,
                        )
                # leftover odd K-tile 42 (j 252..255) — plain matmul, ib10 only
                if ib_hi == NIB:
                    kt = KT - 1
                    ib = NIB - 1
                    w = IB_N[ib]
                    c0 = int(JOFS2[kt]) + 504 * (ib - max(WSPLIT, BMIN[kt]))
                    off = max(0, (JPK * kt + 1 - IB * ib)) * Q
                    nc.tensor.matmul(
                        psums[ib][:, off:w],
                        xo[:, kt * MC + t * 128 : kt * MC + (t + 1) * 128],
                        jt[:, c0 + off : c0 + w],
                        start=False,
                        stop=True,
                    )
                for ib in range(ib_lo, ib_hi):
                    epilogue(t, ib, psums[ib], oh)

            zcols = cpool.tile([128, MT], f32)
            gcols = cpool.tile([128, MT], f32)

            def run_ln(t):
                # ln of this m-tile's 256 softmax denominators and 256 gold
                # exps (2 in-place ACT ops — only the accumulated sums are
                # consumed); the patched table set holds Exp+Ln: no reloads
                nc.scalar.activation(
                    sbig[:, t * L : (t + 1) * L],
                    sbig[:, t * L : (t + 1) * L],
                    mybir.ActivationFunctionType.Ln,
                    accum_out=zcols[:, t : t + 1],
                )
                nc.scalar.activation(
                    gbig[:, t * L : (t + 1) * L],
                    gbig[:, t * L : (t + 1) * L],
                    mybir.ActivationFunctionType.Ln,
                    accum_out=gcols[:, t : t + 1],
                )

            # Wave-2 lags `lag` m-tiles behind wave-1 so the deep K-tiles get
            # extra DMA time before wave-2(0) consumes them.
            ohms = {}
            done = 0
            for t in range(MT):
                ohms[t] = fetch_ohm(t)
                run_wave(t, *WAVES[0], ohms[t])
                for u in o["dues"].get(t, []):
                    assert u == done
                    run_wave(u, *WAVES[1], ohms[u])
                    run_ln(u)
                    done += 1
            while done < MT:
                run_wave(done, *WAVES[1], ohms[done])
                run_ln(done)
                done += 1

            # final combine: per-partition partial of the data NLL
            dm = spool.tile([128, MT], f32, tag="dm")
            nc.vector.tensor_tensor(
                out=dm[:], in0=zcols[:], in1=gcols[:], op=mybir.AluOpType.subtract
            )
            wprod = spool.tile([128, MT], f32, tag="wprod")
            nc.vector.tensor_tensor(
                out=wprod[:], in0=dm[:], in1=wt[:], op=mybir.AluOpType.mult
            )
            # partition-reduce via a tiny f32 matmul against ones so the
            # output DMA is one 4-byte descriptor, not 128 (whose descriptor
            # drain costs ~7us of teardown); gpsimd.partition_all_reduce
            # would swap the Pool ucode library (~6us) instead.
            ones = cpool.tile([128, 1], f32)
            nc.vector.memset(ones[:], 1.0)
            osum = ppool.tile([128, 8], f32, tag="ps", name="osum")
            nc.tensor.matmul(
                osum[0:1, 0:MT], ones[:], wprod[:], start=True, stop=True
            )
            oone = spool.tile([1, 1], f32, tag="oone")
            nc.vector.reduce_sum(
                oone[:], osum[0:1, 0:MT], axis=mybir.AxisListType.X
            )
            nc.sync.dma_start(outd[:], oone[:])

    nc.compile()
    return nc


_GRAPH = None


def _graph():
    global _GRAPH
    if _GRAPH is None:
        _GRAPH = _build_graph()
    return _GRAPH


# ------------------------------------------------------- persistent runner
# Mirrors concourse.bass2jax.run_bass_via_pjrt but caches the jitted
# shard_map executable so repeated calls don't re-trace/re-compile.
class _Runner:
    def __init__(self, nc):
        import jax
        from jax.sharding import Mesh, PartitionSpec
        from jax.experimental.shard_map import shard_map
        import concourse.mybir as mybir
        from concourse import bass2jax

        bass2jax.install_neuronx_cc_hook()
        partition_name = (
            nc.partition_id_tensor.name if nc.partition_id_tensor else None
        )
        in_names, out_names, out_avals, zero_outs = [], [], [], []
        for alloc in nc.m.functions[0].allocations:
            if not isinstance(alloc, mybir.MemoryLocationSet):
                continue
            name = alloc.memorylocations[0].name
            if alloc.kind == "ExternalInput":
                if name != partition_name:
                    in_names.append(name)
            elif alloc.kind == "ExternalOutput":
                shape = tuple(alloc.tensor_shape)
                dtype = mybir.dt.np(alloc.dtype)
                out_names.append(name)
                out_avals.append(jax.core.ShapedArray(shape, dtype))
                zero_outs.append(np.zeros(shape, dtype))
        n_params = len(in_names)
        all_names = in_names + out_names
        if partition_name is not None:
            all_names = all_names + [partition_name]

        def _body(*args):
            operands = list(args)
            if partition_name is not None:
                operands.append(bass2jax.partition_id_tensor())
            outs = bass2jax._bass_exec_p.bind(
                *operands,
                out_avals=tuple(out_avals),
                in_names=tuple(all_names),
                out_names=tuple(out_names),
                lowering_input_output_aliases=(),
                sim_require_finite=True,
                sim_require_nnan=True,
                nc=nc,
            )
            return tuple(outs)

        devices = jax.devices()[:NCORES]
        mesh = Mesh(np.asarray(devices), ("core",))
        self.mesh = mesh
        nin = n_params + len(out_names)
        self._jit = jax.jit(
            shard_map(
                _body,
                mesh=mesh,
                in_specs=(PartitionSpec("core"),) * nin,
                out_specs=(PartitionSpec("core"),) * len(out_names),
                check_rep=False,
            ),
            keep_unused=True,
        )
        self.in_names = in_names
        self.out_names = out_names
        self.out_avals = out_avals
        self.zero_outs = zero_outs
        self._jax = jax

    def put_inputs(self, in_maps, device_resident=True):
        """Concatenate per-core inputs and return the arg list."""
        concat = [
            np.concatenate(
                [np.asarray(in_maps[c][n]) for c in range(NCORES)], axis=0
            )
            for n in self.in_names
        ]
        zeros = [
            np.zeros((NCORES * z.shape[0], *z.shape[1:]), z.dtype)
            for z in self.zero_outs
        ]
        args = concat + zeros
        if device_resident:
            from jax.sharding import NamedSharding, PartitionSpec

            sh = NamedSharding(self.mesh, PartitionSpec("core"))
            args = [self._jax.device_put(a, sh) for a in args]
            self._jax.block_until_ready(args)
        return args

    def run(self, args):
        outs = self._jit(*args)
        self._jax.block_until_ready(outs)
        return {
            n: np.asarray(outs[i]).reshape(NCORES, *self.out_avals[i].shape)
            for i, n in enumerate(self.out_names)
        }


_RUNNER = None


def _runner():
    global _RUNNER
    if _RUNNER is None:
        _RUNNER = _Runner(_graph())
    return _RUNNER


def _make_in_maps(X_idx, W, h_pos, J):
    X_idx = np.asarray(X_idx)
    W = np.asarray(W, np.float32)
    jrs = _prep_shared(J, h_pos)
    in_maps = []
    for c in range(NCORES):
        xoht, ohm, wt = _prep_core(
            X_idx[c * MC : (c + 1) * MC], W[c * MC : (c + 1) * MC]
        )
        in_maps.append({"jrs": jrs, "xoht": xoht, "ohm": ohm, "wt": wt})
    return in_maps


# ---------------------------------------------------------------- entry point
def kernel(X_idx, W, h_pos, J):
    in_maps = _make_in_maps(X_idx, W, h_pos, J)
    reg = _host_reg(J, h_pos)
    try:
        r = _runner()
        out = r.run(r.put_inputs(in_maps))["out"]
    except Exception:
        # stock execution path (slower dispatch, same NEFF)
        res = run_bass_kernel_spmd(
            _graph(), in_maps, core_ids=list(range(NCORES))
        )
        out = np.stack([np.asarray(res.results[c]["out"]) for c in range(NCORES)])
    return np.float32(np.asarray(out, np.float64).sum() + reg)


def bench(X_idx, W, h_pos, J, reps=20):
    """Return (loss, mean_exec_seconds) amortized over reps (incl. RPC)."""
    import time

    reg = _host_reg(J, h_pos)
    r = _runner()
    args = r.put_inputs(_make_in_maps(X_idx, W, h_pos, J))
    out = r.run(args)  # warm-up / compile
    t0 = time.time()
    for _ in range(reps):
        out = r.run(args)
    dt = (time.time() - t0) / reps
    return np.float32(np.asarray(out["out"], np.float64).sum() + reg), dt
